# revision 6
# baseline (speedup 1.0000x reference)
"""Trainium2 Bass kernel for nn_FCGF_RP_AVG (topk masking + masked mean + L2 norm).

Per segment b (64 total, L=50000 points, D=32 feats):
  att = x @ w  (bias dropped: rank-invariant)
  mask = top-1024 of att (threshold via bisection on counts)
  res = sum(mask * x) / L ; out = res / ||res||  (the /L cancels)

Sharding: 8 segments per core x 8 cores, no cross-core comm.

Pipeline (x is read from HBM exactly once; most of it stays resident):
  A. stream x in 25 chunks of 125 points/partition, SWDGE-cast f32->fp16.
     The first 21 chunks land in a persistent SBUF region (xres), the last 4
     in transient tiles (re-fetched cheaply in phase C). att = x.w via one
     tensor_tensor mult + 5-level pairwise add tree (fp16, 2x DVE mode).
  B. threshold bisection: coarse bisection on the first 250 columns
     interleaved with the phase-A stream; widen; full bisection. Counting
     pass = tensor_scalar(is_gt, per-partition threshold broadcast via PE
     matmul, accum_out=count). The final pass writes the select mask as
     int32 {-1, 0} (sign-extended for bitwise masking).
  C. masked sum without any gather/scatter: bitwise_and of the fp16 x pairs
     (viewed as int32) against the {-1,0} mask zeroes unselected points in
     place; then accumulating PE matmuls against segment-indicator columns
     reduce over partitions into a [8, 16, 32] PSUM (position-within-16
     split), a strided reduce folds the 16, and the [8,32] result is
     L2-normalized.
"""

import numpy as np

B = 64
L = 50000
D = 32
TOPK = 1024
NCORES = 8
SEG = B // NCORES          # 8 segments per core
SUB = 16                   # partitions per segment
P = 128
PPTS = L // SUB            # 3125 points per partition
NROW = SEG * L             # 400000 rows per core
CHUNK = 125                # points per partition per chunk
NCHUNK = PPTS // CHUNK     # 25
NRES = 16                  # chunks kept resident in SBUF
FREE = CHUNK * D           # 4000

SUBN = 2 * CHUNK           # 250 leading columns for the coarse bisection
JGRP = 16                  # j-positions folded per accumulate-matmul column

NITER_SUB = 7
NITER_FULL = 7
LO0 = 0.5                  # initial bracket in units of ||w||_2 (att ~ N(0, |w|^2))
HI0 = 3.5
WIDEN = 0.23               # post-coarse widen, units of ||w||_2

_CACHE = {}


def _hoist_sync_waits(nc):
    """Move per-instruction semaphore waits onto standalone EventSemaphore
    instructions. This walrus build rejects instructions whose ISA struct
    lacks enough sync-wait slots (e.g. Tile's kernel-tail Drain)."""
    import bass_rust
    from concourse import mybir

    n = 0
    for bbw in nc.bb_map.values():
        bb = bbw.bb
        new = []
        for inst in bb.instructions:
            si = inst.sync_info
            if si is not None and si.on_wait and not isinstance(
                inst, bass_rust.InstEventSemaphore
            ):
                for k, w in enumerate(si.on_wait):
                    ev = mybir.InstEventSemaphore(
                        name=f"{inst.name}-w{k}", ins=[], outs=[],
                        sync_info=mybir.SyncInfo(on_update=[], on_wait=[w]))
                    ev.engine = inst.engine
                    new.append(ev)
                    n += 1
                inst.sync_info = mybir.SyncInfo(
                    on_update=list(si.on_update), on_wait=[])
            new.append(inst)
        bb.instructions = new
    return n


def _build():
    import concourse.bass as bass
    import concourse.tile as tile
    from concourse import mybir

    nc = bass.Bass()
    f32 = mybir.dt.float32
    f16 = mybir.dt.float16
    i32 = mybir.dt.int32
    Alu = mybir.AluOpType
    Act = mybir.ActivationFunctionType

    x_d = nc.dram_tensor("x", [NROW, D], f32, kind="ExternalInput")
    wrep_d = nc.dram_tensor("wrep", [P, D], f16, kind="ExternalInput")
    blkh_d = nc.dram_tensor("blkh", [P, SEG], f16, kind="ExternalInput")
    blk2_d = nc.dram_tensor("blk2", [P, 2 * SEG], f32, kind="ExternalInput")
    bmid_d = nc.dram_tensor("bmid", [2 * SEG, P], f32, kind="ExternalInput")
    bmid16_d = nc.dram_tensor("bmid16", [2 * SEG, 2 * SEG], f32, kind="ExternalInput")
    sgn_d = nc.dram_tensor("sgn", [2 * SEG, 3], f32, kind="ExternalInput")
    lohi0_d = nc.dram_tensor("lohi0", [2 * SEG, 1], f32, kind="ExternalInput")
    out_d = nc.dram_tensor("out", [SEG, D], f32, kind="ExternalOutput")

    with tile.TileContext(nc) as tc:
        with (
            tc.tile_pool(name="xin", bufs=2) as xin_pool,
            tc.tile_pool(name="work", bufs=1) as work_pool,
            tc.tile_pool(name="persist", bufs=1) as pp,
            tc.tile_pool(name="psum", bufs=1, space="PSUM") as psp,
        ):
            xres = pp.tile([P, NRES * CHUNK, D], f16)   # 125 KiB/partition
            att = pp.tile([P, PPTS], f16)
            attsub = pp.tile([P, SUBN], f16)
            scratch = pp.tile([P, PPTS], f16)
            subscr = pp.tile([P, SUBN], f16)
            m32 = pp.tile([P, PPTS], i32)
            wrep = pp.tile([P, D], f16)
            blkh = pp.tile([P, SEG], f16)
            blk2 = pp.tile([P, 2 * SEG], f32)
            bmid = pp.tile([2 * SEG, P], f32)
            bmid16 = pp.tile([2 * SEG, 2 * SEG], f32)
            sgn = pp.tile([2 * SEG, 3], f32)      # col0: +-1, col1: +-target, col2: widen
            lohi = pp.tile([2 * SEG, 1], f32)     # p<8: lo_s ; p>=8: hi_s

            def xdma(c, dst):
                src = bass.AP(
                    tensor=x_d.tensor if hasattr(x_d, "tensor") else x_d,
                    offset=c * FREE,
                    ap=[[PPTS * D, P], [1, FREE]],
                )
                nc.gpsimd.dma_start(out=dst, in_=src)

            def res_slice(c):
                return xres[:, c * CHUNK:(c + 1) * CHUNK, :]

            # first x chunks ahead of the constants on the Pool queue
            xdma(0, res_slice(0))
            nc.gpsimd.dma_start(out=wrep, in_=wrep_d[:, :])
            xdma(1, res_slice(1))
            nc.gpsimd.dma_start(out=blkh, in_=blkh_d[:, :])
            nc.gpsimd.dma_start(out=blk2, in_=blk2_d[:, :])
            nc.gpsimd.dma_start(out=bmid, in_=bmid_d[:, :])
            nc.gpsimd.dma_start(out=bmid16, in_=bmid16_d[:, :])
            nc.gpsimd.dma_start(out=sgn, in_=sgn_d[:, :])
            nc.gpsimd.dma_start(out=lohi, in_=lohi0_d[:, :])

            # warm-up reads: land the constant-DMA waits on cheap copies so
            # later consumers don't exceed per-instruction sync-wait slots
            warmP = pp.tile([P, 1], f32)
            warm16 = pp.tile([2 * SEG, 1], f32)
            nc.vector.tensor_copy(out=warmP, in_=blk2[:, 0:1])
            nc.vector.tensor_copy(out=warm16, in_=bmid[:, 0:1])
            nc.vector.tensor_copy(out=warm16, in_=bmid16[:, 0:1])
            nc.vector.tensor_copy(out=warm16, in_=sgn[:, 0:1])
            nc.vector.tensor_copy(out=warm16, in_=lohi[:, 0:1])
            wwarm = pp.tile([P, 1], f16)
            nc.vector.tensor_copy(out=wwarm, in_=wrep[:, 0:1])
            nc.vector.tensor_copy(out=wwarm, in_=blkh[:, 0:1])
            # preload the Sqrt activation table off the critical path
            sqwarm = pp.tile([2 * SEG, 1], f32)
            nc.scalar.activation(out=sqwarm, in_=warm16, func=Act.Sqrt)

            # ---- Phase A: att = x.w (fp16 mult + pairwise add tree) ----
            def att_tree(c, xt):
                xw = work_pool.tile([P, CHUNK, D], f16, tag="xw")
                wb = bass.AP(tensor=wrep.tensor, offset=wrep.offset,
                             ap=[wrep.ap[0], [0, CHUNK], [1, D]])
                nc.vector.tensor_tensor(out=xw, in0=xt, in1=wb, op=Alu.mult)
                t16 = work_pool.tile([P, CHUNK, 16], f16, tag="t16")
                nc.vector.tensor_tensor(out=t16, in0=xw[:, :, 0:16], in1=xw[:, :, 16:32], op=Alu.add)
                t8 = work_pool.tile([P, CHUNK, 8], f16, tag="t8")
                nc.vector.tensor_tensor(out=t8, in0=t16[:, :, 0:8], in1=t16[:, :, 8:16], op=Alu.add)
                t4 = work_pool.tile([P, CHUNK, 4], f16, tag="t4")
                nc.vector.tensor_tensor(out=t4, in0=t8[:, :, 0:4], in1=t8[:, :, 4:8], op=Alu.add)
                t2 = work_pool.tile([P, CHUNK, 2], f16, tag="t2")
                nc.vector.tensor_tensor(out=t2, in0=t4[:, :, 0:2], in1=t4[:, :, 2:4], op=Alu.add)
                e0 = bass.AP(tensor=t2.tensor, offset=t2.offset, ap=[t2.ap[0], [2, CHUNK]])
                e1 = bass.AP(tensor=t2.tensor, offset=t2.offset + 1, ap=[t2.ap[0], [2, CHUNK]])
                nc.vector.tensor_tensor(
                    out=att[:, c * CHUNK:(c + 1) * CHUNK], in0=e0, in1=e1, op=Alu.add)

            # ---- bisection machinery ----
            mid128_ps = psp.tile([P, 1], f32, tag="mid128")
            mid16_ps = psp.tile([2 * SEG, 1], f32, tag="mid16")
            segcnt16_ps = psp.tile([2 * SEG, 1], f32, tag="segcnt")
            cnt = pp.tile([P, 1], f32)
            g16 = pp.tile([2 * SEG, 1], i32)
            d16 = pp.tile([2 * SEG, 1], f32)

            def bisect_iter(arr, scr, n, tgt_scale):
                nc.tensor.matmul(out=mid128_ps, lhsT=bmid, rhs=lohi, start=True, stop=True)
                nc.tensor.matmul(out=mid16_ps, lhsT=bmid16, rhs=lohi, start=True, stop=True)
                nc.vector.tensor_scalar(
                    out=scr[:, 0:n], in0=arr[:, 0:n],
                    scalar1=mid128_ps[:, :], scalar2=None, op0=Alu.is_gt, op1=Alu.add,
                    accum_out=cnt)
                nc.tensor.matmul(out=segcnt16_ps, lhsT=blk2, rhs=cnt, start=True, stop=True)
                # d16 = cnt*scale*sgn0 ; g16 = d16 >= sgn1 selects lo/hi updates
                nc.vector.scalar_tensor_tensor(
                    out=d16, in0=segcnt16_ps, scalar=tgt_scale, in1=sgn[:, 0:1],
                    op0=Alu.mult, op1=Alu.mult)
                nc.vector.tensor_scalar(
                    out=g16, in0=d16, scalar1=sgn[:, 1:2], scalar2=None, op0=Alu.is_ge)
                nc.vector.copy_predicated(out=lohi, mask=g16, data=mid16_ps)

            # phase A + coarse bisection interleaved (coarse needs chunks 0,1)
            att_tree(0, res_slice(0))
            att_tree(1, res_slice(1))
            nc.vector.tensor_copy(out=attsub, in_=att[:, 0:SUBN])
            sub_scale = float(L) / (SUBN * SUB)   # subsample count -> full-count units
            it = 0
            for c in range(2, NCHUNK):
                xt = res_slice(c) if c < NRES else xin_pool.tile([P, CHUNK, D], f16, tag="xs")
                xdma(c, xt)
                att_tree(c, xt)
                if c % 3 == 0 and it < NITER_SUB:
                    bisect_iter(attsub, subscr, SUBN, sub_scale)
                    it += 1
            while it < NITER_SUB:
                bisect_iter(attsub, subscr, SUBN, sub_scale)
                it += 1

            # widen bracket by the absolute amount in sgn col2 (+-WIDEN*|w|)
            nc.vector.tensor_scalar(out=lohi, in0=lohi, scalar1=sgn[:, 2:3], scalar2=None, op0=Alu.add)

            for _ in range(NITER_FULL):
                bisect_iter(att, scratch, PPTS, 1.0)

            # prefetch the non-resident chunks for phase C (lands during the
            # final bisection iterations)
            ctiles = []
            for c in range(NRES, NCHUNK):
                xt = xin_pool.tile([P, CHUNK, D], f16, tag="xc")
                xdma(c, xt)
                ctiles.append(xt)

            # final select mask, sign-extended: m32 = (att > t) * -1 in int32
            nc.tensor.matmul(out=mid128_ps, lhsT=bmid, rhs=lohi, start=True, stop=True)
            nc.vector.tensor_scalar(
                out=m32, in0=att, scalar1=mid128_ps[:, :], scalar2=-1.0,
                op0=Alu.is_gt, op1=Alu.mult)

            # ---- Phase C: in-place masking + accumulating matmuls ----
            # Interleave streamed-tail chunks with resident chunks so the
            # tail re-fetch DMAs hide under the resident masking work.
            res_ps = psp.tile([SEG, JGRP * D], f32, tag="res")
            state = {"first": True}

            def and_mask(xt_ap, j0, npts):
                # zero unselected points: fp16 pairs viewed as int32 & {-1,0}
                xi = xt_ap.bitcast(i32)
                mb = bass.AP(tensor=m32.tensor, offset=m32.offset + j0,
                             ap=[m32.ap[0], [1, npts], [0, D // 2]])
                nc.vector.tensor_tensor(out=xi, in0=xi, in1=mb, op=Alu.bitwise_and)

            def accum_mm(flat_ap, nj, stop=False):
                # flat_ap: [P, nj*D] contiguous masked-x; accumulate into
                # res_ps by position within each chunk-local 16-column group
                ngrp = (nj + JGRP - 1) // JGRP
                for g in range(ngrp):
                    n = min(JGRP, nj - g * JGRP) * D
                    ap = bass.AP(tensor=flat_ap.tensor, offset=flat_ap.offset + g * JGRP * D,
                                 ap=[flat_ap.ap[0], [1, n]])
                    nc.tensor.matmul(
                        out=res_ps[:, 0:n], lhsT=blkh, rhs=ap,
                        start=state["first"], stop=stop and g == ngrp - 1)
                    state["first"] = False

            def chunk_c(c, xt_ap, stop=False):
                and_mask(xt_ap, c * CHUNK, CHUNK)
                flat = bass.AP(tensor=xt_ap.tensor, offset=xt_ap.offset,
                               ap=[xt_ap.ap[0], [1, CHUNK * D]])
                accum_mm(flat, CHUNK, stop=stop)

            nstream = len(ctiles)
            order = []
            ri, si = 0, 0
            for c in range(NCHUNK):
                if si < nstream and c % 3 == 2:
                    order.append(("s", si)); si += 1
                elif ri < NRES:
                    order.append(("r", ri)); ri += 1
                else:
                    order.append(("s", si)); si += 1
            for n, (kind, i) in enumerate(order):
                if kind == "r":
                    chunk_c(i, res_slice(i), stop=(n == len(order) - 1))
                else:
                    chunk_c(NRES + i, ctiles[i], stop=(n == len(order) - 1))

            # fold the 16-position split: [8, 16, 32] -> [8, 32]
            psums = pp.tile([SEG, D], f32)
            rview = bass.AP(tensor=res_ps.tensor, offset=res_ps.offset,
                            ap=[res_ps.ap[0], [1, D], [D, JGRP]])
            nc.vector.tensor_reduce(out=psums, in_=rview, axis=mybir.AxisListType.X, op=Alu.add)

            # ---- normalize ----
            sq = pp.tile([SEG, D], f32)
            nrm2 = pp.tile([SEG, 1], f32)
            nrm = pp.tile([SEG, 1], f32)
            rinv = pp.tile([SEG, 1], f32)
            outt = pp.tile([SEG, D], f32)
            nc.vector.scalar_tensor_tensor(
                out=sq, in0=psums, scalar=1.0, in1=psums, op0=Alu.mult, op1=Alu.mult,
                accum_out=nrm2)
            nc.scalar.activation(out=nrm, in_=nrm2, func=Act.Sqrt)
            nc.vector.tensor_scalar(out=nrm, in0=nrm, scalar1=1e-12, scalar2=None, op0=Alu.max)
            nc.vector.reciprocal(out=rinv, in_=nrm)
            nc.vector.tensor_scalar(out=outt, in0=psums, scalar1=rinv[:, :], scalar2=None, op0=Alu.mult)
            nc.sync.dma_start(out=out_d[:, :], in_=outt)

    _hoist_sync_waits(nc)
    return nc


def _constants():
    blk = np.zeros((P, SEG), np.float32)
    for p in range(P):
        blk[p, p // SUB] = 1.0
    blkh = blk.astype(np.float16)
    blk2 = np.concatenate([blk, blk], axis=1)          # [128, 16]
    bmid = np.zeros((2 * SEG, P), np.float32)          # mid128: 0.5(lo_s + hi_s)
    for p in range(P):
        s = p // SUB
        bmid[s, p] = 0.5
        bmid[s + SEG, p] = 0.5
    bmid16 = np.zeros((2 * SEG, 2 * SEG), np.float32)  # mid16: mid of seg q%8
    for q in range(2 * SEG):
        bmid16[q % SEG, q] = 0.5
        bmid16[q % SEG + SEG, q] = 0.5
    return blkh, blk2, bmid, bmid16


def _host_inputs(x, w):
    blkh, blk2, bmid, bmid16 = _CACHE["consts"]
    wrep = np.tile(w[None, :], (P, 1)).astype(np.float16)
    wn = float(np.linalg.norm(w))
    lohi0 = np.array([[LO0 * wn]] * SEG + [[HI0 * wn]] * SEG, np.float32)
    # row q<8 (lo of seg q): move lo up when cnt >= TOPK
    # row q>=8 (hi): move hi down when cnt < TOPK  (-cnt >= -TOPK+0.5)
    sgn = np.zeros((2 * SEG, 3), np.float32)
    sgn[:SEG, 0] = 1.0
    sgn[SEG:, 0] = -1.0
    sgn[:SEG, 1] = float(TOPK)
    sgn[SEG:, 1] = -float(TOPK) + 0.5
    sgn[:SEG, 2] = -WIDEN * wn
    sgn[SEG:, 2] = WIDEN * wn
    in_maps = []
    for i in range(NCORES):
        xs = x[i * NROW:(i + 1) * NROW]
        in_maps.append({"x": xs, "wrep": wrep, "blkh": blkh, "blk2": blk2,
                        "bmid": bmid, "bmid16": bmid16, "sgn": sgn,
                        "lohi0": lohi0})
    return in_maps


def kernel(x, length, w, b):
    from concourse.bass_utils import run_bass_kernel_spmd

    x = np.asarray(x, dtype=np.float32)
    w = np.asarray(w, dtype=np.float32)

    if "nc" not in _CACHE:
        _CACHE["nc"] = _build()
        _CACHE["consts"] = _constants()
    nc = _CACHE["nc"]

    in_maps = _host_inputs(x, w)
    r = run_bass_kernel_spmd(nc, in_maps, list(range(NCORES)))
    out = np.concatenate([r.results[i]["out"] for i in range(NCORES)], axis=0)
    return out.astype(np.float32)


# revision 28
# speedup vs baseline: 1.0076x; 1.0076x over previous
"""Trainium2 Bass kernel for nn_FCGF_RP_AVG (topk masking + masked mean + L2 norm).

Per segment b (64 total, L=50000 points, D=32 feats):
  att = x @ w  (bias dropped: rank-invariant)
  mask = top-1024 of att (threshold via bisection on counts)
  res = sum(mask * x) / L ; out = res / ||res||  (the /L cancels)

Sharding: 8 segments per core x 8 cores, no cross-core comm.

Pipeline (x is read from HBM exactly once; most of it stays resident):
  A. stream x in 25 chunks of 125 points/partition, SWDGE-cast f32->fp16.
     The first 21 chunks land in a persistent SBUF region (xres), the last 4
     in transient tiles (re-fetched cheaply in phase C). att = x.w via one
     tensor_tensor mult + 5-level pairwise add tree (fp16, 2x DVE mode).
  B. threshold bisection: coarse bisection on the first 250 columns
     interleaved with the phase-A stream; widen; full bisection. Counting
     pass = tensor_scalar(is_gt, per-partition threshold broadcast via PE
     matmul, accum_out=count). The final pass writes the select mask as
     int32 {-1, 0} (sign-extended for bitwise masking).
  C. masked sum without any gather/scatter: bitwise_and of the fp16 x pairs
     (viewed as int32) against the {-1,0} mask zeroes unselected points in
     place; then accumulating PE matmuls against segment-indicator columns
     reduce over partitions into a [8, 16, 32] PSUM (position-within-16
     split), a strided reduce folds the 16, and the [8,32] result is
     L2-normalized.
"""

import numpy as np

B = 64
L = 50000
D = 32
TOPK = 1024
NCORES = 8
SEG = B // NCORES          # 8 segments per core
SUB = 16                   # partitions per segment
P = 128
PPTS = L // SUB            # 3125 points per partition
NROW = SEG * L             # 400000 rows per core
CHUNK = 125                # points per partition per chunk
NCHUNK = PPTS // CHUNK     # 25
NRES = 16                  # chunks kept resident in SBUF
FREE = CHUNK * D           # 4000

SUBN = 2 * CHUNK           # 250 leading columns for the coarse bisection
JGRP = 16                  # j-positions folded per accumulate-matmul column

NITER_SUB = 7
NITER_FULL = 7
LO0 = 0.5                  # initial bracket in units of ||w||_2 (att ~ N(0, |w|^2))
HI0 = 3.5
WIDEN = 0.23               # post-coarse widen, units of ||w||_2

_CACHE = {}


def _hoist_sync_waits(nc):
    """Move per-instruction semaphore waits onto standalone EventSemaphore
    instructions. This walrus build rejects instructions whose ISA struct
    lacks enough sync-wait slots (e.g. Tile's kernel-tail Drain)."""
    import bass_rust
    from concourse import mybir

    n = 0
    for bbw in nc.bb_map.values():
        bb = bbw.bb
        new = []
        for inst in bb.instructions:
            si = inst.sync_info
            if si is not None and si.on_wait and not isinstance(
                inst, bass_rust.InstEventSemaphore
            ):
                for k, w in enumerate(si.on_wait):
                    ev = mybir.InstEventSemaphore(
                        name=f"{inst.name}-w{k}", ins=[], outs=[],
                        sync_info=mybir.SyncInfo(on_update=[], on_wait=[w]))
                    ev.engine = inst.engine
                    new.append(ev)
                    n += 1
                inst.sync_info = mybir.SyncInfo(
                    on_update=list(si.on_update), on_wait=[])
            new.append(inst)
        bb.instructions = new
    return n


def _build():
    import concourse.bass as bass
    import concourse.tile as tile
    from concourse import mybir

    nc = bass.Bass()
    f32 = mybir.dt.float32
    f16 = mybir.dt.float16
    i32 = mybir.dt.int32
    Alu = mybir.AluOpType
    Act = mybir.ActivationFunctionType

    x_d = nc.dram_tensor("x", [NROW, D], f32, kind="ExternalInput")
    wrep_d = nc.dram_tensor("wrep", [P, D], f16, kind="ExternalInput")
    blkh_d = nc.dram_tensor("blkh", [P, SEG], f16, kind="ExternalInput")
    blk2_d = nc.dram_tensor("blk2", [P, 2 * SEG], f32, kind="ExternalInput")
    bmid_d = nc.dram_tensor("bmid", [2 * SEG, P], f32, kind="ExternalInput")
    bmid16_d = nc.dram_tensor("bmid16", [2 * SEG, 2 * SEG], f32, kind="ExternalInput")
    sgn_d = nc.dram_tensor("sgn", [2 * SEG, 4], f32, kind="ExternalInput")
    lohi0_d = nc.dram_tensor("lohi0", [2 * SEG, 1], f32, kind="ExternalInput")
    out_d = nc.dram_tensor("out", [SEG, D], f32, kind="ExternalOutput")

    with tile.TileContext(nc) as tc:
        with (
            tc.tile_pool(name="xin", bufs=2) as xin_pool,
            tc.tile_pool(name="work", bufs=1) as work_pool,
            tc.tile_pool(name="persist", bufs=1) as pp,
            tc.tile_pool(name="psum", bufs=1, space="PSUM") as psp,
        ):
            xres = pp.tile([P, NRES * CHUNK, D], f16)   # 125 KiB/partition
            att = pp.tile([P, PPTS], f16)
            attsub = pp.tile([P, SUBN], f16)
            scratch = pp.tile([P, PPTS], f16)
            subscr = pp.tile([P, SUBN], f16)
            m32 = pp.tile([P, PPTS], i32)
            wrep = pp.tile([P, D], f16)
            blkh = pp.tile([P, SEG], f16)
            blk2 = pp.tile([P, 2 * SEG], f32)
            bmid = pp.tile([2 * SEG, P], f32)
            bmid16 = pp.tile([2 * SEG, 2 * SEG], f32)
            sgn = pp.tile([2 * SEG, 4], f32)      # cols: +-1, +-target, widen, +-coarse_scale
            lohi = pp.tile([2 * SEG, 1], f32)     # p<8: lo_s ; p>=8: hi_s

            def xdma(c, dst):
                src = bass.AP(
                    tensor=x_d.tensor if hasattr(x_d, "tensor") else x_d,
                    offset=c * FREE,
                    ap=[[PPTS * D, P], [1, FREE]],
                )
                nc.gpsimd.dma_start(out=dst, in_=src)

            def res_slice(c):
                return xres[:, c * CHUNK:(c + 1) * CHUNK, :]

            # first x sub-chunk (50 cols) ahead of everything so the DVE
            # pipeline starts as early as possible
            def xdma_cols(j0, ncols, dst):
                src = bass.AP(
                    tensor=x_d.tensor if hasattr(x_d, "tensor") else x_d,
                    offset=j0 * D,
                    ap=[[PPTS * D, P], [1, ncols * D]],
                )
                nc.gpsimd.dma_start(out=dst, in_=src)

            xdma_cols(0, 50, xres[:, 0:50, :])
            nc.gpsimd.dma_start(out=wrep, in_=wrep_d[:, :])
            xdma_cols(50, 75, xres[:, 50:125, :])
            xdma(1, res_slice(1))
            nc.gpsimd.dma_start(out=blkh, in_=blkh_d[:, :])
            nc.gpsimd.dma_start(out=blk2, in_=blk2_d[:, :])
            nc.gpsimd.dma_start(out=bmid, in_=bmid_d[:, :])
            nc.gpsimd.dma_start(out=bmid16, in_=bmid16_d[:, :])
            nc.gpsimd.dma_start(out=sgn, in_=sgn_d[:, :])
            nc.gpsimd.dma_start(out=lohi, in_=lohi0_d[:, :])

            # warm-up reads: land the constant-DMA waits on cheap copies so
            # later consumers don't exceed per-instruction sync-wait slots
            warmP = pp.tile([P, 1], f32)
            warm16 = pp.tile([2 * SEG, 1], f32)
            nc.vector.tensor_copy(out=warmP, in_=blk2[:, 0:1])
            nc.vector.tensor_copy(out=warm16, in_=bmid[:, 0:1])
            nc.vector.tensor_copy(out=warm16, in_=bmid16[:, 0:1])
            nc.vector.tensor_copy(out=warm16, in_=sgn[:, 0:1])
            nc.vector.tensor_copy(out=warm16, in_=lohi[:, 0:1])
            wwarm = pp.tile([P, 1], f16)
            nc.vector.tensor_copy(out=wwarm, in_=wrep[:, 0:1])
            nc.vector.tensor_copy(out=wwarm, in_=blkh[:, 0:1])
            # preload the Sqrt activation table off the critical path
            sqwarm = pp.tile([2 * SEG, 1], f32)
            nc.scalar.activation(out=sqwarm, in_=warm16, func=Act.Sqrt)

            # ---- Phase A: att = x.w (fp16 mult + pairwise add tree) ----
            def tree_ops(j0, npts, xt):
                xw_t = work_pool.tile([P, CHUNK, D], f16, tag="xw")
                xw = xw_t[:, 0:npts, :]
                t16_t = work_pool.tile([P, CHUNK, 16], f16, tag="t16")
                t16 = t16_t[:, 0:npts, :]
                t8_t = work_pool.tile([P, CHUNK, 8], f16, tag="t8")
                t8 = t8_t[:, 0:npts, :]
                t4_t = work_pool.tile([P, CHUNK, 4], f16, tag="t4")
                t4 = t4_t[:, 0:npts, :]
                t2_t = work_pool.tile([P, CHUNK, 2], f16, tag="t2")
                t2 = t2_t[:, 0:npts, :]
                wb = bass.AP(tensor=wrep.tensor, offset=wrep.offset,
                             ap=[wrep.ap[0], [0, npts], [1, D]])
                e0 = bass.AP(tensor=t2.tensor, offset=t2.offset, ap=[t2.ap[0], [2, npts]])
                e1 = bass.AP(tensor=t2.tensor, offset=t2.offset + 1, ap=[t2.ap[0], [2, npts]])
                return [
                    lambda: nc.vector.tensor_tensor(out=xw, in0=xt, in1=wb, op=Alu.mult),
                    lambda: nc.vector.tensor_tensor(out=t16, in0=xw[:, :, 0:16], in1=xw[:, :, 16:32], op=Alu.add),
                    lambda: nc.vector.tensor_tensor(out=t8, in0=t16[:, :, 0:8], in1=t16[:, :, 8:16], op=Alu.add),
                    lambda: nc.vector.tensor_tensor(out=t4, in0=t8[:, :, 0:4], in1=t8[:, :, 4:8], op=Alu.add),
                    lambda: nc.vector.tensor_tensor(out=t2, in0=t4[:, :, 0:2], in1=t4[:, :, 2:4], op=Alu.add),
                    lambda: nc.vector.tensor_tensor(out=att[:, j0:j0 + npts], in0=e0, in1=e1, op=Alu.add),
                ]

            def att_tree_n(j0, npts, xt):
                for op in tree_ops(j0, npts, xt):
                    op()

            def att_tree(c, xt):
                att_tree_n(c * CHUNK, CHUNK, xt)

            def att_tree_pair(c1, xt1, c2, xt2):
                # zip two chunks' trees so every op's producer is 2 slots
                # back in the in-order DVE queue (hides RAW sem latency)
                a = tree_ops(c1 * CHUNK, CHUNK, xt1)
                b = tree_ops(c2 * CHUNK, CHUNK, xt2)
                for oa, ob in zip(a, b):
                    oa()
                    ob()

            # ---- bisection machinery ----
            mid128_ps = psp.tile([P, 1], f32, tag="mid128")
            mid16_ps = psp.tile([2 * SEG, 1], f32, tag="mid16")
            segcnt16_ps = psp.tile([2 * SEG, 1], f32, tag="segcnt")
            cnt = pp.tile([P, 1], f32)
            g16 = pp.tile([2 * SEG, 1], i32)

            def bisect_iter(arr, scr, n, scol, update=True):
                nc.tensor.matmul(out=mid128_ps, lhsT=bmid, rhs=lohi, start=True, stop=True)
                nc.tensor.matmul(out=mid16_ps, lhsT=bmid16, rhs=lohi, start=True, stop=True)
                nc.vector.tensor_scalar(
                    out=scr[:, 0:n], in0=arr[:, 0:n],
                    scalar1=mid128_ps[:, :], scalar2=None, op0=Alu.is_gt, op1=Alu.add,
                    accum_out=cnt)
                if not update:
                    return
                nc.tensor.matmul(out=segcnt16_ps, lhsT=blk2, rhs=cnt, start=True, stop=True)
                # g16 = (cnt * (+-scale)) >= +-target selects lo/hi updates
                nc.vector.tensor_scalar(
                    out=g16, in0=segcnt16_ps, scalar1=sgn[:, scol:scol + 1],
                    scalar2=sgn[:, 1:2], op0=Alu.mult, op1=Alu.is_ge)
                nc.vector.copy_predicated(out=lohi, mask=g16, data=mid16_ps)

            # phase A + coarse bisection interleaved (coarse needs chunks 0,1)
            att_tree_n(0, 50, xres[:, 0:50, :])
            att_tree_n(50, 75, xres[:, 50:125, :])
            att_tree(1, res_slice(1))
            nc.vector.tensor_copy(out=attsub, in_=att[:, 0:SUBN])
            it = 0
            for c in range(2, NCHUNK):
                xt = res_slice(c) if c < NRES else xin_pool.tile([P, CHUNK, D], f16, tag="xs")
                xdma(c, xt)
                att_tree(c, xt)
                if c % 3 == 0 and it < NITER_SUB:
                    bisect_iter(attsub, subscr, SUBN, 3)
                    it += 1
            while it < NITER_SUB:
                bisect_iter(attsub, subscr, SUBN, 3)
                it += 1

            # widen bracket by the absolute amount in sgn col2 (+-WIDEN*|w|)
            nc.vector.tensor_scalar(out=lohi, in0=lohi, scalar1=sgn[:, 2:3], scalar2=None, op0=Alu.add)

            for i in range(NITER_FULL):
                bisect_iter(att, scratch, PPTS, 0, update=(i < NITER_FULL - 1))

            # prefetch the non-resident chunks for phase C (lands during the
            # final bisection iterations)
            ctiles = []
            for c in range(NRES, NCHUNK):
                xt = xin_pool.tile([P, CHUNK, D], f16, tag="xc")
                xdma(c, xt)
                ctiles.append(xt)

            # final select mask, sign-extended: m32 = (att > t) * -1 in int32
            # (two halves so the first phase-C ANDs can start sooner)
            HALF = PPTS // 2
            nc.tensor.matmul(out=mid128_ps, lhsT=bmid, rhs=lohi, start=True, stop=True)
            nc.vector.tensor_scalar(
                out=m32[:, 0:HALF], in0=att[:, 0:HALF], scalar1=mid128_ps[:, :],
                scalar2=-1.0, op0=Alu.is_gt, op1=Alu.mult)
            nc.vector.tensor_scalar(
                out=m32[:, HALF:PPTS], in0=att[:, HALF:PPTS], scalar1=mid128_ps[:, :],
                scalar2=-1.0, op0=Alu.is_gt, op1=Alu.mult)

            # ---- Phase C: in-place masking + accumulating matmuls ----
            # Interleave streamed-tail chunks with resident chunks so the
            # tail re-fetch DMAs hide under the resident masking work.
            res_ps = psp.tile([SEG, JGRP * D], f32, tag="res")
            state = {"first": True}

            def and_mask(xt_ap, j0, npts):
                # zero unselected points: fp16 pairs viewed as int32 & {-1,0}
                xi = xt_ap.bitcast(i32)
                mb = bass.AP(tensor=m32.tensor, offset=m32.offset + j0,
                             ap=[m32.ap[0], [1, npts], [0, D // 2]])
                nc.vector.tensor_tensor(out=xi, in0=xi, in1=mb, op=Alu.bitwise_and)

            def accum_mm(flat_ap, nj, stop=False):
                # flat_ap: [P, nj*D] contiguous masked-x; accumulate into
                # res_ps by position within each chunk-local 16-column group
                ngrp = (nj + JGRP - 1) // JGRP
                for g in range(ngrp):
                    n = min(JGRP, nj - g * JGRP) * D
                    ap = bass.AP(tensor=flat_ap.tensor, offset=flat_ap.offset + g * JGRP * D,
                                 ap=[flat_ap.ap[0], [1, n]])
                    nc.tensor.matmul(
                        out=res_ps[:, 0:n], lhsT=blkh, rhs=ap,
                        start=state["first"], stop=stop and g == ngrp - 1)
                    state["first"] = False

            def chunk_c(c, xt_ap, stop=False):
                and_mask(xt_ap, c * CHUNK, CHUNK)
                flat = bass.AP(tensor=xt_ap.tensor, offset=xt_ap.offset,
                               ap=[xt_ap.ap[0], [1, CHUNK * D]])
                accum_mm(flat, CHUNK, stop=stop)

            nstream = len(ctiles)
            order = []
            ri, si = 0, 0
            for c in range(NCHUNK):
                if si < nstream and c % 3 == 2:
                    order.append(("s", si)); si += 1
                elif ri < NRES:
                    order.append(("r", ri)); ri += 1
                else:
                    order.append(("s", si)); si += 1
            for n, (kind, i) in enumerate(order):
                if kind == "r":
                    chunk_c(i, res_slice(i), stop=(n == len(order) - 1))
                else:
                    chunk_c(NRES + i, ctiles[i], stop=(n == len(order) - 1))

            # fold the 16-position split: [8, 16, 32] -> [8, 32]
            psums = pp.tile([SEG, D], f32)
            rview = bass.AP(tensor=res_ps.tensor, offset=res_ps.offset,
                            ap=[res_ps.ap[0], [1, D], [D, JGRP]])
            nc.vector.tensor_reduce(out=psums, in_=rview, axis=mybir.AxisListType.X, op=Alu.add)

            # ---- normalize ----
            sq = pp.tile([SEG, D], f32)
            nrm2 = pp.tile([SEG, 1], f32)
            nrm = pp.tile([SEG, 1], f32)
            rinv = pp.tile([SEG, 1], f32)
            outt = pp.tile([SEG, D], f32)
            nc.vector.scalar_tensor_tensor(
                out=sq, in0=psums, scalar=1.0, in1=psums, op0=Alu.mult, op1=Alu.mult,
                accum_out=nrm2)
            nc.scalar.activation(out=nrm, in_=nrm2, func=Act.Sqrt)
            nc.vector.reciprocal(out=rinv, in_=nrm)
            nc.vector.tensor_scalar(out=outt, in0=psums, scalar1=rinv[:, :], scalar2=None, op0=Alu.mult)
            nc.sync.dma_start(out=out_d[:, :], in_=outt)

    _hoist_sync_waits(nc)
    return nc


def _constants():
    blk = np.zeros((P, SEG), np.float32)
    for p in range(P):
        blk[p, p // SUB] = 1.0
    blkh = blk.astype(np.float16)
    blk2 = np.concatenate([blk, blk], axis=1)          # [128, 16]
    bmid = np.zeros((2 * SEG, P), np.float32)          # mid128: 0.5(lo_s + hi_s)
    for p in range(P):
        s = p // SUB
        bmid[s, p] = 0.5
        bmid[s + SEG, p] = 0.5
    bmid16 = np.zeros((2 * SEG, 2 * SEG), np.float32)  # mid16: mid of seg q%8
    for q in range(2 * SEG):
        bmid16[q % SEG, q] = 0.5
        bmid16[q % SEG + SEG, q] = 0.5
    return blkh, blk2, bmid, bmid16


def _host_inputs(x, w):
    blkh, blk2, bmid, bmid16 = _CACHE["consts"]
    wrep = np.tile(w[None, :], (P, 1)).astype(np.float16)
    wn = float(np.linalg.norm(w))
    lohi0 = np.array([[LO0 * wn]] * SEG + [[HI0 * wn]] * SEG, np.float32)
    # row q<8 (lo of seg q): move lo up when cnt >= TOPK
    # row q>=8 (hi): move hi down when cnt < TOPK  (-cnt >= -TOPK+0.5)
    sgn = np.zeros((2 * SEG, 4), np.float32)
    sgn[:SEG, 0] = 1.0
    sgn[SEG:, 0] = -1.0
    sgn[:SEG, 1] = float(TOPK)
    sgn[SEG:, 1] = -float(TOPK) + 0.5
    sgn[:SEG, 2] = -WIDEN * wn
    sgn[SEG:, 2] = WIDEN * wn
    cs = float(L) / (SUBN * SUB)
    sgn[:SEG, 3] = cs
    sgn[SEG:, 3] = -cs
    in_maps = []
    for i in range(NCORES):
        xs = x[i * NROW:(i + 1) * NROW]
        in_maps.append({"x": xs, "wrep": wrep, "blkh": blkh, "blk2": blk2,
                        "bmid": bmid, "bmid16": bmid16, "sgn": sgn,
                        "lohi0": lohi0})
    return in_maps


def kernel(x, length, w, b):
    from concourse.bass_utils import run_bass_kernel_spmd

    x = np.asarray(x, dtype=np.float32)
    w = np.asarray(w, dtype=np.float32)

    if "nc" not in _CACHE:
        _CACHE["nc"] = _build()
        _CACHE["consts"] = _constants()
    nc = _CACHE["nc"]

    in_maps = _host_inputs(x, w)
    r = run_bass_kernel_spmd(nc, in_maps, list(range(NCORES)))
    out = np.concatenate([r.results[i]["out"] for i in range(NCORES)], axis=0)
    return out.astype(np.float32)
